# revision 5
# baseline (speedup 1.0000x reference)
"""MoE layer (T=16384, H=1024, F=4096, E=8, top-2) on 8 Trainium2 cores.

Strategy (expert parallelism, per the sharding hint):
  - Router (x @ Wg, softmax, top-2, renormalize) runs on host with jax-on-CPU
    so expert selection matches the reference bit-for-bit. This is the
    "dispatch" half of the all-to-all: it decides which tokens go in which
    core's input map.
  - Core e receives: xeT  = gathered tokens of expert e, transposed [H, C] bf16
                     w1   = w1[e] bf16 [H, F]
                     w2   = w2[e] bf16 [F, H]
    and computes yeT = (silu(xe @ w1) @ w2)^T entirely on-device with
    bf16 matmuls accumulating in fp32 PSUM. Weights stay resident in SBUF.
  - Host combine ("all-to-all return"): y[idx_e] += gate_e[:,None] * ye.
"""

import numpy as np
import ml_dtypes

T, H, F, E, TOPK = 16384, 1024, 4096, 8, 2
P = 128
CHUNK = 256           # matmul moving free dim (tokens per chunk)
KT = H // P           # 8  k-tiles over H
FT = F // P           # 32 tiles over F
HT = H // P           # 8  output tiles over H

BF16 = ml_dtypes.bfloat16

_module_cache: dict = {}


def _routing(x: np.ndarray, Wg: np.ndarray):
    """Top-2 expert ids and renormalized gates, matching the jax reference.

    The reference receives numpy arrays, so its `x @ Wg` runs through numpy
    BLAS — replicate that exactly (the expert ranking has 1-ulp knife-edge
    ties that flip between BLAS and XLA matmul). softmax/top_k then follow
    the reference's jax ops on CPU.
    """
    logits = x @ Wg  # numpy BLAS fp32, same as reference(**np_inputs)
    try:
        import jax
        import jax.numpy as jnp

        cpu = jax.devices("cpu")[0]
        with jax.default_device(cpu):
            lj = jax.device_put(jnp.asarray(logits), cpu)
            probs = jax.nn.softmax(lj, axis=-1)
            tv, ti = jax.lax.top_k(probs, TOPK)
            rw = tv / jnp.sum(tv, axis=-1, keepdims=True)
        return np.asarray(ti), np.asarray(rw, np.float32)
    except Exception:
        m = logits.max(axis=1, keepdims=True)
        p = np.exp(logits - m)
        p /= p.sum(axis=1, keepdims=True)
        order = np.argsort(-p, axis=1, kind="stable")
        ti = order[:, :TOPK]
        tv = np.take_along_axis(p, ti, axis=1)
        rw = (tv / tv.sum(axis=1, keepdims=True)).astype(np.float32)
        return ti, rw


def _build_module(C: int):
    """Bass/Tile module for one expert-core: yeT = (silu(xe@w1)@w2)^T."""
    import concourse.bass as bass
    import concourse.mybir as mybir
    import concourse.tile as tile
    from concourse import bacc
    from concourse.bass import ts

    dt = mybir.dt
    n_chunks = C // CHUNK

    nc = bacc.Bacc("TRN2", target_bir_lowering=False, debug=False)

    xeT = nc.dram_tensor("xeT", (KT, P, C), dt.bfloat16, kind="ExternalInput").ap()
    w1 = nc.dram_tensor("w1", (KT, P, F), dt.bfloat16, kind="ExternalInput").ap()
    w2 = nc.dram_tensor("w2", (FT, P, H), dt.bfloat16, kind="ExternalInput").ap()
    yeT = nc.dram_tensor("yeT", (HT, P, C), dt.float32, kind="ExternalOutput").ap()

    with tile.TileContext(nc) as tc:
        with (
            tc.tile_pool(name="wpool", bufs=1) as wpool,
            tc.tile_pool(name="xpool", bufs=2) as xpool,
            tc.tile_pool(name="hpool", bufs=2) as hpool,
            tc.tile_pool(name="opool", bufs=4) as opool,
            tc.tile_pool(name="spool", bufs=4) as spool,
            tc.tile_pool(name="ps1", bufs=3, space="PSUM") as ps1,
            tc.tile_pool(name="ps2", bufs=3, space="PSUM") as ps2,
        ):
            # Resident weights: 64KB + 64KB per partition.
            w1s = wpool.tile([P, KT, F], dt.bfloat16)
            w2s = wpool.tile([P, FT, H], dt.bfloat16)
            for k in range(KT):
                nc.sync.dma_start(out=w1s[:, k, :], in_=w1[k, :, :])
            for f in range(FT):
                nc.sync.dma_start(out=w2s[:, f, :], in_=w2[f, :, :])

            for j in range(n_chunks):
                xt = xpool.tile([P, KT, CHUNK], dt.bfloat16, tag="xt")
                for k in range(KT):
                    nc.sync.dma_start(
                        out=xt[:, k, :], in_=xeT[k, :, ts(j, CHUNK)]
                    )
                ht = hpool.tile([P, FT, CHUNK], dt.bfloat16, tag="ht")
                for f in range(FT):
                    ph = ps1.tile([P, CHUNK], dt.float32, tag="ph")
                    for k in range(KT):
                        nc.tensor.matmul(
                            ph[:],
                            lhsT=w1s[:, k, ts(f, P)],
                            rhs=xt[:, k, :],
                            start=(k == 0),
                            stop=(k == KT - 1),
                        )
                    # silu(x) = x * sigmoid(x); HW Silu LUT set is broken on
                    # this runtime (NRT_EXEC_UNIT_UNRECOVERABLE), so compose it.
                    sg = spool.tile([P, CHUNK], dt.float32, tag="sg")
                    nc.scalar.activation(
                        sg[:], ph[:], mybir.ActivationFunctionType.Sigmoid
                    )
                    nc.vector.tensor_mul(ht[:, f, :], sg[:], ph[:])
                for h in range(HT):
                    py = ps2.tile([P, CHUNK], dt.float32, tag="py")
                    for f in range(FT):
                        nc.tensor.matmul(
                            py[:],
                            lhsT=w2s[:, f, ts(h, P)],
                            rhs=ht[:, f, :],
                            start=(f == 0),
                            stop=(f == FT - 1),
                        )
                    ot = opool.tile([P, CHUNK], dt.float32, tag="ot")
                    nc.vector.tensor_copy(ot[:], py[:])
                    nc.sync.dma_start(out=yeT[h, :, ts(j, CHUNK)], in_=ot[:])

    nc.compile()
    return nc


def _get_module(C: int):
    if C not in _module_cache:
        _module_cache[C] = _build_module(C)
    return _module_cache[C]


def kernel(x: np.ndarray, Wg: np.ndarray, w1: np.ndarray, w2: np.ndarray,
           **_unused) -> np.ndarray:
    from concourse.bass_utils import run_bass_kernel_spmd

    x = np.ascontiguousarray(np.asarray(x, np.float32))
    Wg = np.asarray(Wg, np.float32)
    w1 = np.asarray(w1, np.float32)
    w2 = np.asarray(w2, np.float32)
    nt = x.shape[0]

    ti, rw = _routing(x, Wg)

    # Per-expert token index lists + gates (dispatch).
    idx_list, gate_list = [], []
    for e in range(E):
        hit = ti == e                                   # [nt, 2]
        rows = np.nonzero(hit.any(axis=1))[0]
        g = np.where(hit[rows, 0], rw[rows, 0], rw[rows, 1]).astype(np.float32)
        idx_list.append(rows)
        gate_list.append(g)

    max_cnt = max(len(r) for r in idx_list)
    C = max(CHUNK, ((max_cnt + CHUNK - 1) // CHUNK) * CHUNK)

    nc = _get_module(C)

    in_maps = []
    for e in range(E):
        rows = idx_list[e]
        xeT = np.zeros((H, C), BF16)
        xeT[:, : len(rows)] = x[rows].T.astype(BF16)
        in_maps.append(
            {
                "xeT": xeT.reshape(KT, P, C),
                "w1": w1[e].astype(BF16).reshape(KT, P, F),
                "w2": w2[e].astype(BF16).reshape(FT, P, H),
            }
        )

    res = run_bass_kernel_spmd(nc, in_maps, core_ids=list(range(E)))

    # Combine (the return half of the all-to-all) with gate scaling.
    y = np.zeros((nt, H), np.float32)
    for e in range(E):
        rows = idx_list[e]
        ye = res.results[e]["yeT"].reshape(H, C)[:, : len(rows)]
        y[rows] += gate_list[e][:, None] * ye.T
    return y


if __name__ == "__main__":
    rng = np.random.default_rng(0)
    xs = rng.standard_normal((T, H), dtype=np.float32)
    Wgs = rng.standard_normal((H, E), dtype=np.float32) / np.sqrt(H)
    w1s = rng.standard_normal((E, H, F), dtype=np.float32) / np.sqrt(H)
    w2s = rng.standard_normal((E, F, H), dtype=np.float32) / np.sqrt(F)
    out = kernel(x=xs, Wg=Wgs, w1=w1s, w2=w2s)
    print(out.shape, out.dtype)


# revision 8
# speedup vs baseline: 67.7408x; 67.7408x over previous
"""MoE layer (T=16384, H=1024, F=4096, E=8, top-2) on 8 Trainium2 cores.

Strategy (expert parallelism, per the sharding hint):
  - Router (x @ Wg, softmax, top-2, renormalize) runs on host with jax-on-CPU
    so expert selection matches the reference bit-for-bit. This is the
    "dispatch" half of the all-to-all: it decides which tokens go in which
    core's input map.
  - Core e receives: xeT  = gathered tokens of expert e, transposed [H, C] bf16
                     w1   = w1[e] bf16 [H, F]
                     w2   = w2[e] bf16 [F, H]
    and computes yeT = (silu(xe @ w1) @ w2)^T entirely on-device with
    bf16 matmuls accumulating in fp32 PSUM. Weights stay resident in SBUF.
  - Host combine ("all-to-all return"): y[idx_e] += gate_e[:,None] * ye.
"""

import numpy as np
import ml_dtypes

T, H, F, E, TOPK = 16384, 1024, 4096, 8, 2
P = 128
CHUNK = 256           # matmul moving free dim (tokens per chunk)
KT = H // P           # 8  k-tiles over H
FT = F // P           # 32 tiles over F
HT = H // P           # 8  output tiles over H

BF16 = ml_dtypes.bfloat16

_module_cache: dict = {}


def _routing(x: np.ndarray, Wg: np.ndarray):
    """Top-2 expert ids and renormalized gates, matching the jax reference.

    The reference receives numpy arrays, so its `x @ Wg` runs through numpy
    BLAS — replicate that exactly (the expert ranking has 1-ulp knife-edge
    ties that flip between BLAS and XLA matmul). softmax/top_k then follow
    the reference's jax ops on CPU.
    """
    logits = x @ Wg  # numpy BLAS fp32, same as reference(**np_inputs)
    try:
        import jax
        import jax.numpy as jnp

        cpu = jax.devices("cpu")[0]
        with jax.default_device(cpu):
            lj = jax.device_put(jnp.asarray(logits), cpu)
            probs = jax.nn.softmax(lj, axis=-1)
            tv, ti = jax.lax.top_k(probs, TOPK)
            rw = tv / jnp.sum(tv, axis=-1, keepdims=True)
        return np.asarray(ti), np.asarray(rw, np.float32)
    except Exception:
        m = logits.max(axis=1, keepdims=True)
        p = np.exp(logits - m)
        p /= p.sum(axis=1, keepdims=True)
        order = np.argsort(-p, axis=1, kind="stable")
        ti = order[:, :TOPK]
        tv = np.take_along_axis(p, ti, axis=1)
        rw = (tv / tv.sum(axis=1, keepdims=True)).astype(np.float32)
        return ti, rw


def _build_module(C: int, repeat: int = 1):
    """Bass/Tile module for one expert-core: yeT = (silu(xe@w1)@w2)^T.

    repeat>1 re-runs the whole token loop (same I/O) for differential
    benchmarking — wall(R) - wall(1) = (R-1) * device_time.
    """
    import concourse.bass as bass
    import concourse.mybir as mybir
    import concourse.tile as tile
    from concourse import bacc
    from concourse.bass import ts

    dt = mybir.dt
    n_chunks = C // CHUNK

    nc = bacc.Bacc("TRN2", target_bir_lowering=False, debug=False)

    xeT = nc.dram_tensor("xeT", (KT, P, C), dt.bfloat16, kind="ExternalInput").ap()
    w1 = nc.dram_tensor("w1", (KT, P, F), dt.bfloat16, kind="ExternalInput").ap()
    w2 = nc.dram_tensor("w2", (FT, P, H), dt.bfloat16, kind="ExternalInput").ap()
    yeT = nc.dram_tensor("yeT", (HT, P, C), dt.float32, kind="ExternalOutput").ap()

    with tile.TileContext(nc) as tc:
        with (
            tc.tile_pool(name="wpool", bufs=1) as wpool,
            tc.tile_pool(name="xpool", bufs=2) as xpool,
            tc.tile_pool(name="hpool", bufs=2) as hpool,
            tc.tile_pool(name="opool", bufs=4) as opool,
            tc.tile_pool(name="spool", bufs=4) as spool,
            tc.tile_pool(name="ps1", bufs=3, space="PSUM") as ps1,
            tc.tile_pool(name="ps2", bufs=3, space="PSUM") as ps2,
        ):
            # Resident weights: 64KB + 64KB per partition.
            w1s = wpool.tile([P, KT, F], dt.bfloat16)
            w2s = wpool.tile([P, FT, H], dt.bfloat16)
            for k in range(KT):
                nc.sync.dma_start(out=w1s[:, k, :], in_=w1[k, :, :])
            for f in range(FT):
                nc.sync.dma_start(out=w2s[:, f, :], in_=w2[f, :, :])

            for j_rep in range(n_chunks * repeat):
                j = j_rep % n_chunks
                xt = xpool.tile([P, KT, CHUNK], dt.bfloat16, tag="xt")
                for k in range(KT):
                    nc.sync.dma_start(
                        out=xt[:, k, :], in_=xeT[k, :, ts(j, CHUNK)]
                    )
                ht = hpool.tile([P, FT, CHUNK], dt.bfloat16, tag="ht")
                for f in range(FT):
                    ph = ps1.tile([P, CHUNK], dt.float32, tag="ph")
                    for k in range(KT):
                        nc.tensor.matmul(
                            ph[:],
                            lhsT=w1s[:, k, ts(f, P)],
                            rhs=xt[:, k, :],
                            start=(k == 0),
                            stop=(k == KT - 1),
                        )
                    # silu(x) = x * sigmoid(x); HW Silu LUT set is broken on
                    # this runtime (NRT_EXEC_UNIT_UNRECOVERABLE), so compose it.
                    sg = spool.tile([P, CHUNK], dt.float32, tag="sg")
                    nc.scalar.activation(
                        sg[:], ph[:], mybir.ActivationFunctionType.Sigmoid
                    )
                    nc.vector.tensor_mul(ht[:, f, :], sg[:], ph[:])
                for h in range(HT):
                    py = ps2.tile([P, CHUNK], dt.float32, tag="py")
                    for f in range(FT):
                        nc.tensor.matmul(
                            py[:],
                            lhsT=w2s[:, f, ts(h, P)],
                            rhs=ht[:, f, :],
                            start=(f == 0),
                            stop=(f == FT - 1),
                        )
                    ot = opool.tile([P, CHUNK], dt.float32, tag="ot")
                    nc.vector.tensor_copy(ot[:], py[:])
                    nc.sync.dma_start(out=yeT[h, :, ts(j, CHUNK)], in_=ot[:])

    nc.compile()
    return nc


def _get_module(C: int, repeat: int = 1):
    key = (C, repeat)
    if key not in _module_cache:
        _module_cache[key] = _build_module(C, repeat)
    return _module_cache[key]


def kernel(x: np.ndarray, Wg: np.ndarray, w1: np.ndarray, w2: np.ndarray,
           **_unused) -> np.ndarray:
    from concourse.bass_utils import run_bass_kernel_spmd

    x = np.ascontiguousarray(np.asarray(x, np.float32))
    Wg = np.asarray(Wg, np.float32)
    w1 = np.asarray(w1, np.float32)
    w2 = np.asarray(w2, np.float32)
    nt = x.shape[0]

    ti, rw = _routing(x, Wg)

    # Per-expert token index lists + gates (dispatch).
    idx_list, gate_list = [], []
    for e in range(E):
        hit = ti == e                                   # [nt, 2]
        rows = np.nonzero(hit.any(axis=1))[0]
        g = np.where(hit[rows, 0], rw[rows, 0], rw[rows, 1]).astype(np.float32)
        idx_list.append(rows)
        gate_list.append(g)

    max_cnt = max(len(r) for r in idx_list)
    C = max(CHUNK, ((max_cnt + CHUNK - 1) // CHUNK) * CHUNK)

    nc = _get_module(C)

    in_maps = []
    for e in range(E):
        rows = idx_list[e]
        xeT = np.zeros((H, C), BF16)
        xeT[:, : len(rows)] = x[rows].T.astype(BF16)
        in_maps.append(
            {
                "xeT": xeT.reshape(KT, P, C),
                "w1": w1[e].astype(BF16).reshape(KT, P, F),
                "w2": w2[e].astype(BF16).reshape(FT, P, H),
            }
        )

    res = run_bass_kernel_spmd(nc, in_maps, core_ids=list(range(E)))

    # Combine (the return half of the all-to-all) with gate scaling.
    y = np.zeros((nt, H), np.float32)
    for e in range(E):
        rows = idx_list[e]
        ye = res.results[e]["yeT"].reshape(H, C)[:, : len(rows)]
        y[rows] += gate_list[e][:, None] * ye.T
    return y


if __name__ == "__main__":
    rng = np.random.default_rng(0)
    xs = rng.standard_normal((T, H), dtype=np.float32)
    Wgs = rng.standard_normal((H, E), dtype=np.float32) / np.sqrt(H)
    w1s = rng.standard_normal((E, H, F), dtype=np.float32) / np.sqrt(H)
    w2s = rng.standard_normal((E, F, H), dtype=np.float32) / np.sqrt(F)
    out = kernel(x=xs, Wg=Wgs, w1=w1s, w2=w2s)
    print(out.shape, out.dtype)


# revision 15
# speedup vs baseline: 91.5523x; 1.3515x over previous
"""MoE layer (T=16384, H=1024, F=4096, E=8, top-2) on 8 Trainium2 cores.

Strategy (expert parallelism, per the sharding hint):
  - Router (x @ Wg, softmax, top-2, renormalize) runs on host with jax-on-CPU
    so expert selection matches the reference bit-for-bit. This is the
    "dispatch" half of the all-to-all: it decides which tokens go in which
    core's input map.
  - Core e receives: xeT  = gathered tokens of expert e, transposed [H, C] bf16
                     w1   = w1[e] bf16 [H, F]
                     w2   = w2[e] bf16 [F, H]
    and computes yeT = (silu(xe @ w1) @ w2)^T entirely on-device with
    bf16 matmuls accumulating in fp32 PSUM. Weights stay resident in SBUF.
  - Host combine ("all-to-all return"): y[idx_e] += gate_e[:,None] * ye.
"""

import numpy as np
import ml_dtypes

T, H, F, E, TOPK = 16384, 1024, 4096, 8, 2
P = 128
CHUNK = 512           # matmul moving free dim (tokens per chunk)
KT = H // P           # 8  k-tiles over H
FT = F // P           # 32 tiles over F
HT = H // P           # 8  output tiles over H

BF16 = ml_dtypes.bfloat16

_module_cache: dict = {}


def _routing(x: np.ndarray, Wg: np.ndarray):
    """Top-2 expert ids and renormalized gates, matching the jax reference.

    The reference receives numpy arrays, so its `x @ Wg` runs through numpy
    BLAS — replicate that exactly (the expert ranking has 1-ulp knife-edge
    ties that flip between BLAS and XLA matmul). softmax/top_k then follow
    the reference's jax ops on CPU.
    """
    logits = x @ Wg  # numpy BLAS fp32, same as reference(**np_inputs)
    try:
        import jax
        import jax.numpy as jnp

        cpu = jax.devices("cpu")[0]
        with jax.default_device(cpu):
            lj = jax.device_put(jnp.asarray(logits), cpu)
            probs = jax.nn.softmax(lj, axis=-1)
            tv, ti = jax.lax.top_k(probs, TOPK)
            rw = tv / jnp.sum(tv, axis=-1, keepdims=True)
        return np.asarray(ti), np.asarray(rw, np.float32)
    except Exception:
        m = logits.max(axis=1, keepdims=True)
        p = np.exp(logits - m)
        p /= p.sum(axis=1, keepdims=True)
        order = np.argsort(-p, axis=1, kind="stable")
        ti = order[:, :TOPK]
        tv = np.take_along_axis(p, ti, axis=1)
        rw = (tv / tv.sum(axis=1, keepdims=True)).astype(np.float32)
        return ti, rw


def _build_module(C: int, repeat: int = 1):
    """Bass/Tile module for one expert-core: yeT = (silu(xe@w1)@w2)^T.

    repeat>1 re-runs the whole token loop (same I/O) for differential
    benchmarking — wall(R) - wall(1) = (R-1) * device_time.
    """
    import concourse.bass as bass
    import concourse.mybir as mybir
    import concourse.tile as tile
    from concourse import bacc
    from concourse.bass import ts

    dt = mybir.dt
    n_chunks = C // CHUNK

    nc = bacc.Bacc("TRN2", target_bir_lowering=False, debug=False)

    xeT = nc.dram_tensor("xeT", (KT, P, C), dt.bfloat16, kind="ExternalInput").ap()
    w1 = nc.dram_tensor("w1", (KT, P, F), dt.bfloat16, kind="ExternalInput").ap()
    w2 = nc.dram_tensor("w2", (FT, P, H), dt.bfloat16, kind="ExternalInput").ap()
    yeT = nc.dram_tensor("yeT", (HT, P, C), dt.float32, kind="ExternalOutput").ap()

    with tile.TileContext(nc) as tc:
        with (
            tc.tile_pool(name="wpool", bufs=1) as wpool,
            tc.tile_pool(name="xpool", bufs=2) as xpool,
            tc.tile_pool(name="hpool", bufs=1) as hpool,
            tc.tile_pool(name="opool", bufs=2) as opool,
            tc.tile_pool(name="spool", bufs=2) as spool,
            tc.tile_pool(name="ps1", bufs=4, space="PSUM") as ps1,
            tc.tile_pool(name="ps2", bufs=4, space="PSUM") as ps2,
        ):
            # Resident weights: 64KB + 64KB per partition.
            w1s = wpool.tile([P, KT, F], dt.bfloat16)
            w2s = wpool.tile([P, FT, H], dt.bfloat16)
            for k in range(KT):
                nc.sync.dma_start(out=w1s[:, k, :], in_=w1[k, :, :])
            for f in range(FT):
                nc.sync.dma_start(out=w2s[:, f, :], in_=w2[f, :, :])

            for j_rep in range(n_chunks * repeat):
                j = j_rep % n_chunks
                xt = xpool.tile([P, KT, CHUNK], dt.bfloat16, tag="xt")
                for k in range(KT):
                    nc.sync.dma_start(
                        out=xt[:, k, :], in_=xeT[k, :, ts(j, CHUNK)]
                    )
                ht = hpool.tile([P, FT, CHUNK], dt.bfloat16, tag="ht")
                for f in range(FT):
                    ph = ps1.tile([P, CHUNK], dt.float32, tag="ph")
                    for k in range(KT):
                        nc.tensor.matmul(
                            ph[:],
                            lhsT=w1s[:, k, ts(f, P)],
                            rhs=xt[:, k, :],
                            start=(k == 0),
                            stop=(k == KT - 1),
                        )
                    # silu(x) = x * sigmoid(x); HW Silu LUT set is broken on
                    # this runtime (NRT_EXEC_UNIT_UNRECOVERABLE), so compose it.
                    sg = spool.tile([P, CHUNK], dt.float32, tag="sg")
                    nc.scalar.activation(
                        sg[:], ph[:], mybir.ActivationFunctionType.Sigmoid
                    )
                    nc.vector.tensor_mul(ht[:, f, :], sg[:], ph[:])
                for h in range(HT):
                    py = ps2.tile([P, CHUNK], dt.float32, tag="py")
                    for f in range(FT):
                        nc.tensor.matmul(
                            py[:],
                            lhsT=w2s[:, f, ts(h, P)],
                            rhs=ht[:, f, :],
                            start=(f == 0),
                            stop=(f == FT - 1),
                        )
                    ot = opool.tile([P, CHUNK], dt.float32, tag="ot")
                    nc.vector.tensor_copy(ot[:], py[:])
                    nc.sync.dma_start(out=yeT[h, :, ts(j, CHUNK)], in_=ot[:])

    nc.compile()
    return nc


def _get_module(C: int, repeat: int = 1):
    key = (C, repeat)
    if key not in _module_cache:
        _module_cache[key] = _build_module(C, repeat)
    return _module_cache[key]


def kernel(x: np.ndarray, Wg: np.ndarray, w1: np.ndarray, w2: np.ndarray,
           **_unused) -> np.ndarray:
    from concourse.bass_utils import run_bass_kernel_spmd

    x = np.ascontiguousarray(np.asarray(x, np.float32))
    Wg = np.asarray(Wg, np.float32)
    w1 = np.asarray(w1, np.float32)
    w2 = np.asarray(w2, np.float32)
    nt = x.shape[0]

    ti, rw = _routing(x, Wg)

    # Per-expert token index lists + gates (dispatch).
    idx_list, gate_list = [], []
    for e in range(E):
        hit = ti == e                                   # [nt, 2]
        rows = np.nonzero(hit.any(axis=1))[0]
        g = np.where(hit[rows, 0], rw[rows, 0], rw[rows, 1]).astype(np.float32)
        idx_list.append(rows)
        gate_list.append(g)

    max_cnt = max(len(r) for r in idx_list)
    C = max(CHUNK, ((max_cnt + CHUNK - 1) // CHUNK) * CHUNK)

    nc = _get_module(C)

    in_maps = []
    for e in range(E):
        rows = idx_list[e]
        xeT = np.zeros((H, C), BF16)
        xeT[:, : len(rows)] = x[rows].T.astype(BF16)
        in_maps.append(
            {
                "xeT": xeT.reshape(KT, P, C),
                "w1": w1[e].astype(BF16).reshape(KT, P, F),
                "w2": w2[e].astype(BF16).reshape(FT, P, H),
            }
        )

    res = run_bass_kernel_spmd(nc, in_maps, core_ids=list(range(E)))

    # Combine (the return half of the all-to-all) with gate scaling.
    y = np.zeros((nt, H), np.float32)
    for e in range(E):
        rows = idx_list[e]
        ye = res.results[e]["yeT"].reshape(H, C)[:, : len(rows)]
        y[rows] += gate_list[e][:, None] * ye.T
    return y


if __name__ == "__main__":
    rng = np.random.default_rng(0)
    xs = rng.standard_normal((T, H), dtype=np.float32)
    Wgs = rng.standard_normal((H, E), dtype=np.float32) / np.sqrt(H)
    w1s = rng.standard_normal((E, H, F), dtype=np.float32) / np.sqrt(H)
    w2s = rng.standard_normal((E, F, H), dtype=np.float32) / np.sqrt(F)
    out = kernel(x=xs, Wg=Wgs, w1=w1s, w2=w2s)
    print(out.shape, out.dtype)
